# revision 20
# baseline (speedup 1.0000x reference)
"""Soft decision-tree forward (nn_DTree) on 8 trn2 NeuronCores.

Strategy (pure data parallel, per the sharding hint):
  - shard x row-wise 8 ways; replicate the tiny tree params.
  - HOST pre-transposes x into a fold-3 layout xt3 [128, 11008] bf16:
    rows are split into 3 contiguous classes (11008/10880/10880 rows);
    class s puts [x | 1 | 1]^T (34 rows: 32 features + two bias-ones
    rows for the bf16-split threshold c_hi/c_lo) at partitions
    34s..34s+33.  This makes the input DMA a single cheap [128, C]
    transfer and removes all on-device transposes.
  - per core, per 128-row tile: z = [x|1|1] @ [W | -c_hi | -c_lo] via a
    bf16 PE matmul into fp32 PSUM (lhsT = xt3 slice, rhs = weights
    replicated at each class partition base), g = sigmoid(z) on ACT,
    then a level-by-level value-tree blend:
       V_k = g_k * (V_{k+1,L} - V_{k+1,R}) + V_{k+1,R}
    with nodes pre-permuted level-major so each level's children are
    two contiguous halves of the previous level.
  - engine split: the level-7 fold (v7 = g7*delta + beta, ~40% of the
    blend element count) runs on GPSIMD; levels 6..0 run on DVE; both
    overlap with ACT (sigmoid) and PE (matmuls) across slot-groups.
"""

import numpy as np
import ml_dtypes

import concourse.bass as bass
import concourse.bacc as bacc
import concourse.tile as tile
from concourse import mybir
from concourse.bass_utils import run_bass_kernel_spmd

BF16 = ml_dtypes.bfloat16

F = 32
D = 8
NODES = 255
LEAVES = 256
N_FULL = 262144
N_CORES = 8
ROWS = N_FULL // N_CORES  # 32768 rows per core
TILES = ROWS // 128       # 256 tiles per core

# fold-2 row classes: contiguous row halves; PE requires operand base
# partition in {0, 32, 64}, so the two 34-row feature groups sit at 0 and 64
CLS_TILES = (128, 128)
CLS_START = (0, 128)                # first tile index of each class
CLS_BASE = (0, 16384)               # first row of each class
CLS_PBASE = (0, 64)                 # SBUF partition base of each class
XT_COLS = 16384

# level-major offsets of each level's gates inside the 255-column block
LEVEL_OFF = {7: 0, 6: 128, 5: 192, 4: 224, 3: 240, 2: 248, 1: 252, 0: 254}


def _orderings():
    """ord[k] = local node order at level k (left-children-first recursion)."""
    ordv = {0: [0]}
    for k in range(7):
        ordv[k + 1] = [2 * i for i in ordv[k]] + [2 * i + 1 for i in ordv[k]]
    col_nodes = []
    for k in range(7, -1, -1):
        base = 2 ** k - 1
        col_nodes += [base + i for i in ordv[k]]
    return ordv, np.array(col_nodes)


def host_prep(feature_importances, feature_splits, leaf_node_classes, slots):
    """Tiny-param preprocessing (O(8K) work): relu/sigmoid/c, node
    permutation, bf16 weight matrix with split bias rows replicated at the
    3 class partition bases, leaf-blend constants."""
    fi = np.asarray(feature_importances, np.float32).reshape(NODES, F)
    fs = np.asarray(feature_splits, np.float32).reshape(NODES, F)
    cls = np.asarray(leaf_node_classes, np.float32).reshape(LEAVES)

    W = np.maximum(fi, 0.0)
    S = 1.0 / (1.0 + np.exp(-fs))
    c = np.sum(W * S, axis=1)  # (NODES,)

    ordv, col_nodes = _orderings()
    Wp = W[col_nodes]          # (255, 32) permuted level-major
    cp = c[col_nodes]

    c_hi = cp.astype(BF16).astype(np.float32)
    c_lo = (cp - c_hi).astype(np.float32)

    wt = np.zeros((128, 256), BF16)
    for s in range(2):  # replicate for both row-class partition groups
        b = 64 * s
        wt[b : b + F, 0:NODES] = Wp.T.astype(BF16)
        wt[b + F, 0:NODES] = (-c_hi).astype(BF16)
        wt[b + F + 1, 0:NODES] = (-c_lo).astype(BF16)

    o7 = np.array(ordv[7])
    delta = (cls[2 * o7] - cls[2 * o7 + 1]).astype(BF16)
    beta = cls[2 * o7 + 1].astype(BF16)
    # (node, slot) layout: value for node j replicated across `slots` columns
    db = np.zeros((128, 2 * slots * 128), BF16)
    db[:, : slots * 128] = np.repeat(delta, slots)[None, :]
    db[:, slots * 128 :] = np.repeat(beta, slots)[None, :]
    db8 = np.zeros((128, 2 * 8 * 128), BF16)
    db8[:, : 8 * 128] = np.repeat(delta, 8)[None, :]
    db8[:, 8 * 128 :] = np.repeat(beta, 8)[None, :]
    return wt, db, db8


def host_xt3(x_core):
    """[32768, 32] f32 -> fold-3 transposed bf16 [128, XT_COLS]."""
    xe = np.empty((ROWS, 34), BF16)
    xe[:, 0:F] = x_core.astype(BF16)
    xe[:, F : F + 2] = 1
    xt3 = np.zeros((128, XT_COLS), BF16)
    for s in range(2):
        r0 = CLS_BASE[s]
        r1 = CLS_BASE[s + 1] if s < 1 else ROWS
        xt3[64 * s : 64 * s + 34, 0 : r1 - r0] = xe[r0:r1].T
    return xt3


def out_permutation():
    """Physical row index for each (partition p, device-output column t)."""
    perm = np.empty((128, TILES), np.int64)
    p = np.arange(128)
    for t in range(TILES):
        s = 0 if t < CLS_START[1] else 1
        perm[:, t] = CLS_BASE[s] + (t - CLS_START[s]) * 128 + p
    return perm


def _cls_of(t):
    return 0 if t < CLS_START[1] else 1


def build_nc(slots=32, pw=8, gbufs=4, xchunks=8, osplit=8, stage=4, gp_v7=True, gpattern=None, cf=1, cl=0, gp6=0, zbf=False, dbslot=2):
    """Build the single-core Bass program (SPMD across the cores).

    stage: 1=DMA only, 2=+matmuls, 3=+sigmoid, 4=full (blend+output).
    gp_v7: run the level-7 fold on GPSIMD (else DVE).
    """
    assert TILES % slots == 0 and slots % pw == 0
    groups = TILES // slots
    bf = mybir.dt.bfloat16
    f32 = mybir.dt.float32
    Act = mybir.ActivationFunctionType
    Alu = mybir.AluOpType

    nc = bacc.Bacc()
    xt_in = nc.dram_tensor("xt", [128, XT_COLS], bf, kind="ExternalInput")
    wt_in = nc.dram_tensor("wt", [128, 256], bf, kind="ExternalInput")
    db_in = nc.dram_tensor("db", [128, 2 * slots * 128], bf, kind="ExternalInput")
    db8_in = nc.dram_tensor("db8", [128, 2 * 8 * 128], bf, kind="ExternalInput")
    out_dram = nc.dram_tensor("out", [128, TILES], bf, kind="ExternalOutput")

    with tile.TileContext(nc) as tc:
        with (
            tc.tile_pool(name="consts", bufs=1) as consts,
            tc.tile_pool(name="xsb", bufs=1) as xsb,
            tc.tile_pool(name="zps", bufs=2, space="PSUM") as zps,
            tc.tile_pool(name="gpool", bufs=gbufs) as gpool,
            tc.tile_pool(name="vpool", bufs=2) as vpool,
            tc.tile_pool(name="dpool", bufs=2) as dpool,
            tc.tile_pool(name="opool", bufs=1) as opool,
        ):
            # ---- constants + input (xt first: it gates the PE pipeline) ----
            wt_sb = consts.tile([128, 256], bf)
            nc.sync.dma_start(out=wt_sb[:], in_=wt_in[:])

            # x^T fold-2: straight chunked DMA, no transposes.  The leaf-blend
            # constants ride between the first chunks: early enough for the
            # first group's level-7 fold, without gating the PE start.
            xt = xsb.tile([128, XT_COLS], bf)
            dbt = consts.tile([128, 2 * slots * 128], bf)
            db8t = consts.tile([128, 2 * 8 * 128], bf)
            cw = XT_COLS // xchunks
            for ci in range(xchunks):
                sl = slice(ci * cw, XT_COLS if ci == xchunks - 1 else (ci + 1) * cw)
                nc.sync.dma_start(out=xt[:, sl], in_=xt_in[:, sl])
                if ci == 0:
                    # tiny 8-slot leaf consts: unblocks the first DVE chunk early
                    nc.sync.dma_start(out=db8t[:], in_=db8_in[:])
                if ci == dbslot:
                    nc.sync.dma_start(out=dbt[:], in_=db_in[:])
            dbc = dbt[:, 0 : slots * 128]
            bbc = dbt[:, slots * 128 :]

            out_sb = opool.tile([128, TILES], bf)

            if stage == 1:
                nc.gpsimd.dma_start(out=out_dram[:, 0:1], in_=xt[0:128, 0:1])

            # variable group sizes: small edge groups start the blend engines
            # early and shrink the final drain, without per-op FD overhead
            gsizes = gpattern or [slots] * (TILES // slots)
            assert sum(gsizes) == TILES and all(g % pw == 0 and g <= slots for g in gsizes)
            gbase = [sum(gsizes[:i]) for i in range(len(gsizes))]

            for gi in range(len(gsizes) if stage >= 2 else 0):
                gsz = gsizes[gi]
                # g layout: [128, node, slot] — every tree slice is a flat
                # contiguous range, keeping DVE in the bf16 2x perf mode.
                g_t = gpool.tile([128, 256, slots], bf, tag="g_t")
                for w in range(gsz // pw):
                    zt = zps.tile([128, pw * 256], bf if zbf else f32, tag="zt")
                    ztv = zt[:].rearrange("p (j c) -> p c j", c=256)
                    for j in range(pw):
                        t = gbase[gi] + w * pw + j
                        s = _cls_of(t)
                        c0 = (t - CLS_START[s]) * 128
                        b = CLS_PBASE[s]
                        nc.tensor.matmul(
                            ztv[:, 0:NODES, j],
                            lhsT=xt[b : b + 34, c0 : c0 + 128],
                            rhs=wt_sb[b : b + 34, 0:NODES],
                            start=True,
                            stop=True,
                        )
                    if stage == 2:
                        continue
                    nc.scalar.activation(
                        out=g_t[:, 0:NODES, w * pw : (w + 1) * pw],
                        in_=ztv[:, 0:NODES, :],
                        func=Act.Sigmoid,
                    )
                if stage == 2:
                    continue
                if stage == 3:
                    nc.vector.tensor_copy(
                        out_sb[:, gbase[gi] : gbase[gi] + gsz], g_t[:, 0, 0:gsz]
                    )
                    continue

                # ---- value tree ----
                # [node, slot] 3D views so the blend can run on slot sub-ranges
                dbc3 = dbc.rearrange("p (n s) -> p n s", s=slots)
                bbc3 = bbc.rearrange("p (n s) -> p n s", s=slots)
                dbc83 = db8t[:, 0 : 8 * 128].rearrange("p (n s) -> p n s", s=8)
                bbc83 = db8t[:, 8 * 128 :].rearrange("p (n s) -> p n s", s=8)
                eng7 = nc.gpsimd if gp_v7 else nc.vector

                def blend(s0, s1, ob=gbase[gi], g_t=g_t, gi=gi):
                    v = vpool.tile([128, 128, slots], bf, tag="v7")
                    if gi == 0 and s0 == 0 and s1 - s0 == 8:
                        # first chunk: v7 on DVE from the tiny db8 so the blend
                        # starts without waiting for GPSIMD or the big db
                        nc.vector.tensor_tensor(
                            v[:, :, 0:8], g_t[:, 0:128, 0:8], dbc83[:], Alu.mult
                        )
                        nc.vector.tensor_tensor(
                            v[:, :, 0:8], v[:, :, 0:8], bbc83[:], Alu.add
                        )
                    else:
                        eng7.tensor_tensor(
                            v[:, :, s0:s1], g_t[:, 0:128, s0:s1], dbc3[:, :, s0:s1],
                            Alu.mult,
                        )
                        eng7.tensor_tensor(
                            v[:, :, s0:s1], v[:, :, s0:s1], bbc3[:, :, s0:s1],
                            Alu.add,
                        )
                    for k in range(6, -1, -1):
                        m = 2 ** k
                        off = LEVEL_OFF[k]
                        vl = v[:, 0:m, s0:s1]
                        vr = v[:, m : 2 * m, s0:s1]
                        dt = dpool.tile([128, 64, slots], bf, tag="dtmp")
                        dts = dt[:, 0:m, s0:s1]
                        eng_s = nc.gpsimd if (k == 6 and cf <= gi < cf + gp6) else nc.vector
                        eng_s.tensor_tensor(dts, vl, vr, Alu.subtract)
                        gk = g_t[:, off : off + m, s0:s1]
                        if k > 0:
                            vn = vpool.tile([128, 64, slots], bf, tag="vn")
                            vns = vn[:, 0:m, s0:s1]
                            nc.vector.tensor_mul(vns, gk, dts)
                            nc.vector.tensor_add(vns, vns, vr)
                            v = vn
                        else:
                            vo = out_sb[:, ob + s0 : ob + s1]
                            gk2 = g_t[:, off, s0:s1]
                            nc.vector.tensor_mul(vo, gk2, dt[:, 0, s0:s1])
                            nc.vector.tensor_add(vo, vo, v[:, 1, s0:s1])

                if gi < cf or gi >= len(gsizes) - cl:
                    # sub-chunk per ACT wave: early groups so the blend
                    # engines start sooner, late groups to shrink the drain
                    for s0 in range(0, gsz, pw):
                        blend(s0, s0 + pw)
                else:
                    blend(0, gsz)

            if stage >= 3:
                step = max(1, TILES // osplit)
                for c0 in range(0, TILES, step):
                    c1 = min(c0 + step, TILES)
                    nc.sync.dma_start(out=out_dram[:, c0:c1], in_=out_sb[:, c0:c1])
    return nc


_CACHE = {}


def _get_nc(slots=32, pw=8):
    key = (slots, pw)
    if key not in _CACHE:
        nc = build_nc(slots=slots, pw=pw)
        if not nc.is_finalized():
            nc.finalize()
        _CACHE[key] = nc
    return _CACHE[key]


def run_device(x, wt, db, db8, slots=32, pw=8, n_cores=N_CORES, trace=False):
    rows = x.shape[0] // n_cores
    assert rows == ROWS
    nc = _get_nc(slots, pw)
    in_maps = [
        {
            "xt": host_xt3(x[i * rows : (i + 1) * rows]),
            "wt": wt,
            "db": db,
            "db8": db8,
        }
        for i in range(n_cores)
    ]
    res = run_bass_kernel_spmd(nc, in_maps, list(range(n_cores)), trace=trace)
    perm = out_permutation()
    out = np.empty((n_cores * rows, 1), np.float32)
    for i in range(n_cores):
        dev = res.results[i]["out"].astype(np.float32)  # [128, TILES]
        core_out = np.empty(rows, np.float32)
        core_out[perm.ravel()] = dev.ravel()
        out[i * rows : (i + 1) * rows, 0] = core_out
    return out, res


def kernel(**inputs):
    x = np.asarray(inputs["x"], np.float32).reshape(-1, F)
    slots = 32
    wt, db, db8 = host_prep(
        inputs["feature_importances"],
        inputs["feature_splits"],
        inputs["leaf_node_classes"],
        slots,
    )
    out, _ = run_device(x, wt, db, db8, slots)
    return out
